# revision 13
# baseline (speedup 1.0000x reference)
"""External-attention kernel for 8 Trainium2 NeuronCores.

Reference computation (per batch b, token t):
    q      = x @ Wq.T + bq
    scores = q @ mem.T
    w      = softmax(scores)
    att    = w @ mem
    out    = att @ Wo.T + bo + x

Because the memory bank is tiny (256 slots) the projections are folded
into it on the host (exact algebra, done in float64):
    Keff = (mem @ Wq).T          # [E, M]
    s0   = mem @ bq - bo @ Keff  # [M]      (bias fold, xb = x + bo below)
    Veff = mem @ Wo.T            # [M, E]
    xb   = x + bo
    scores = xb @ Keff + s0
    out    = softmax(scores) @ Veff + xb
This is a 5x FLOP reduction vs. the reference graph.

Sharding: data-parallel over batch (8 batches -> 8 cores), weights
replicated. On-chip everything lives feature-major ([E|M, tokens]); the
host pre-permutes x into a chunked partition-major layout so every DMA
descriptor moves 16 KiB contiguous per partition, and un-permutes the
output.

All matmuls run as float32r (fp32 storage, FP22 compute) which streams
at 1 row/cycle on the PE like bf16 but with 13 mantissa bits.
"""

import os
import sys

import numpy as np

if not any(os.path.isdir(os.path.join(p, "concourse")) for p in sys.path if p):
    sys.path.insert(0, "/opt/trn_rl_repo")

import concourse.bass as bass
import concourse.mybir as mybir
import concourse.tile as tile
from concourse import bacc
from concourse import bass_utils
from concourse.bass import ts

F32 = mybir.dt.float32
F16 = mybir.dt.float16
F32R = mybir.dt.float32r

E = 1024          # embed dim
M = 256           # memory slots
B = 8             # batch (== number of cores)
T = 4096          # tokens per core
CHUNK = 512       # tokens processed per pipeline step
N_CHUNKS = T // CHUNK
ET = E // 128     # e-tiles (8)
MT = M // 128     # m-tiles (2)
TT = CHUNK // 128 # 128-token tiles per chunk (4)

N_CORES = 8

# Of the ET out-tiles per chunk, this many are evicted PSUM->SBUF by a
# fused DVE add; the rest go ACT-copy + DVE SBUF-mode add (engine balance).
DVE_FUSED_TILES = 5

# Module-level switches (test.py pokes these).
TRACE = False
LAST_RESULTS = None

_CACHE = {}

_AXON_SO = "/opt/axon/libaxon_pjrt.so"


def _ntff_hook_via_ctypes(so_path):
    """(output_dir, device_ids) -> contextmanager driving NTFF capture via
    the axon PJRT .so's C ABI. Mirrors trn_boot._ntff_profile_via_ctypes."""
    import contextlib
    import ctypes

    lib = ctypes.CDLL(so_path)
    if not hasattr(lib, "axon_start_nrt_profile"):
        return None
    lib.axon_start_nrt_profile.argtypes = [
        ctypes.POINTER(ctypes.c_int64),
        ctypes.c_size_t,
    ]
    lib.axon_start_nrt_profile.restype = ctypes.c_int64
    lib.axon_stop_nrt_profile.argtypes = [ctypes.c_char_p]
    lib.axon_stop_nrt_profile.restype = ctypes.c_int64

    @contextlib.contextmanager
    def _hook(output_dir, device_ids):
        import jax

        jax.devices()
        if device_ids:
            ids = (ctypes.c_int64 * len(device_ids))(*device_ids)
            rc = lib.axon_start_nrt_profile(ids, len(device_ids))
        else:
            rc = lib.axon_start_nrt_profile(None, 0)
        if rc != 0:
            raise RuntimeError(f"axon_start_nrt_profile rc={rc}")
        try:
            yield
        finally:
            n = lib.axon_stop_nrt_profile(str(output_dir).encode())
            print(f"ntff profile: {n} file(s) written to {output_dir}",
                  file=sys.stderr)

    return _hook


def _ensure_trace_support():
    """Make trace=True survive environments missing antenv.axon_hooks or
    artifact-share access. No-ops where the real plumbing exists."""
    try:
        import antenv.axon_hooks  # noqa: F401
    except ImportError:
        import types

        import antenv

        mod = types.ModuleType("antenv.axon_hooks")
        holder = {"hook": None}
        mod.set_axon_ntff_profile_hook = lambda h: holder.__setitem__("hook", h)
        mod.get_axon_ntff_profile_hook = lambda: holder["hook"]
        antenv.axon_hooks = mod
        sys.modules["antenv.axon_hooks"] = mod
        if os.path.exists(_AXON_SO):
            try:
                hook = _ntff_hook_via_ctypes(_AXON_SO)
                if hook is not None:
                    mod.set_axon_ntff_profile_hook(hook)
            except Exception:
                pass

    if not getattr(bass_utils.upload_artifacts, "_safe", False):
        orig = bass_utils.upload_artifacts

        def safe_upload(tmpdir):
            try:
                return orig(tmpdir)
            except Exception:
                return f"local:{tmpdir}"

        safe_upload._safe = True
        bass_utils.upload_artifacts = safe_upload


def _build_kernel():
    nc = bacc.Bacc(
        "TRN2",
        target_bir_lowering=False,
        debug=False,
        num_devices=N_CORES,
    )

    # x / out in chunked partition-major layout: [c, p, a, t] holds
    # element (token c*CHUNK+t, embed a*128+p). Each (c, p) block is a
    # contiguous run -> large DMA descriptors. x rides in fp16: it only
    # feeds the scores matmul (the fp32 residual is applied on the host).
    xbt = nc.dram_tensor(
        "xbt", [N_CHUNKS, 128, ET, CHUNK], F16, kind="ExternalInput"
    ).ap()
    # Weights pre-packed partition-major on the host: one contiguous run
    # per partition.
    keff = nc.dram_tensor("keff", [128, ET, M], F16, kind="ExternalInput").ap()
    veff = nc.dram_tensor("veff", [128, MT, E], F16, kind="ExternalInput").ap()
    s0 = nc.dram_tensor("s0", [1, M], F32, kind="ExternalInput").ap()
    ones = nc.dram_tensor("ones", [1, 128], F32, kind="ExternalInput").ap()
    ident = nc.dram_tensor("ident", [128, 128], F16, kind="ExternalInput").ap()
    outt = nc.dram_tensor(
        "outt", [N_CHUNKS, 128, ET, CHUNK], F16, kind="ExternalOutput"
    ).ap()

    with tile.TileContext(nc) as tc:
        with (
            tc.tile_pool(name="const", bufs=1) as const,
            tc.tile_pool(name="xin", bufs=3) as xin,
            tc.tile_pool(name="soft", bufs=3) as soft,
            tc.tile_pool(name="ptw", bufs=3) as ptw,
            tc.tile_pool(name="stats", bufs=8) as stats,
            tc.tile_pool(name="ostage", bufs=3) as ostage,
            tc.tile_pool(name="ps_sc", bufs=2, space="PSUM") as ps_sc_pool,
            tc.tile_pool(name="ps_tr", bufs=2, space="PSUM") as ps_tr_pool,
            tc.tile_pool(name="ps_out", bufs=4, space="PSUM") as ps_out_pool,
        ):
            # All loads share the sync ring, emitted in dependency-priority
            # order: chunk-0 x, then the constants that gate its compute,
            # then later chunks. Stores ride the scalar ring in parallel.
            xt0 = xin.tile([128, ET, CHUNK], F16, tag="xt")
            s0_sb = const.tile([1, M], F32R)
            nc.sync.dma_start(s0_sb[:], s0.bitcast(F32R))
            ones_sb = const.tile([1, 128], F32R)
            nc.sync.dma_start(ones_sb[:], ones.bitcast(F32R))
            keff_sb = const.tile([128, ET, M], F16)
            nc.sync.dma_start(keff_sb[:], keff)
            # Chunk 0 arrives t_tile-sliced so scores(t0) can start after
            # ~1/4 of the chunk landed.
            for tt in range(TT):
                nc.sync.dma_start(
                    xt0[:, :, ts(tt, 128)], xbt[0][:, :, ts(tt, 128)]
                )
            id_f16 = const.tile([128, 128], F16)
            nc.sync.dma_start(id_f16[:], ident)
            veff_sb = const.tile([128, MT, E], F16)
            nc.sync.dma_start(veff_sb[:], veff)
            # Touch Exp once so the ACT table load happens during the
            # initial DMAs, not on chunk 0's critical path.
            warm = const.tile([1, 1], F32)
            nc.scalar.activation(
                warm[:], s0_sb[:1, :1].bitcast(F32),
                mybir.ActivationFunctionType.Exp,
            )

            def emit_chunk(c, toff, ntok, xt_pre=None):
                ntt = ntok // 128
                if xt_pre is not None:
                    xt = xt_pre
                else:
                    xt = xin.tile([128, ET, ntok], F16, tag="xt")
                    nc.sync.dma_start(
                        xt[:], xbt[c][:, :, bass.ds(toff, ntok)]
                    )

                # Transposed softmax weights for this span: [m, t].
                pt_sb = ptw.tile([128, MT, ntok], F16, tag="pt")

                # Phase 1: scores matmuls, with each t_tile's transposes
                # emitted one scores-group later (softmax latency hiding).
                pns = []
                trans_done = 0

                def emit_transpose(tt):
                    for mt in range(MT):
                        ptp = ps_tr_pool.tile([128, 128], F16)
                        nc.tensor.transpose(
                            ptp[:], pns[tt][:, ts(mt, 128)], id_f16[:]
                        )
                        nc.vector.tensor_copy(
                            out=pt_sb[:, mt, ts(tt, 128)], in_=ptp[:]
                        )

                for tt in range(ntt):
                    # scores[t, m] = xb @ Keff + s0  (PSUM accumulate)
                    sc = ps_sc_pool.tile([128, M], F32)
                    nc.tensor.matmul(
                        sc[:], ones_sb[:], s0_sb[:],
                        start=True, stop=False,
                    )
                    for e in range(ET):
                        nc.tensor.matmul(
                            sc[:],
                            xt[:, e, ts(tt, 128)],
                            keff_sb[:, e, :],
                            start=False, stop=(e == ET - 1),
                        )

                    # softmax over m (free dim)
                    negmx = stats.tile([128, 1], F32)
                    nc.vector.reduce_max(
                        negmx[:], sc[:], axis=mybir.AxisListType.X, negate=True
                    )
                    p_sb = soft.tile([128, M], F32, tag="p_exp")
                    sums = stats.tile([128, 1], F32)
                    nc.scalar.activation(
                        p_sb[:], sc[:], mybir.ActivationFunctionType.Exp,
                        bias=negmx[:], scale=1.0, accum_out=sums[:],
                    )
                    rsum = stats.tile([128, 1], F32)
                    nc.vector.reciprocal(rsum[:], sums[:])
                    pn = soft.tile([128, M], F16, tag=f"p_norm_{tt}")
                    nc.scalar.activation(
                        pn[:], p_sb[:], mybir.ActivationFunctionType.Copy,
                        scale=rsum[:],
                    )
                    pns.append(pn)
                    if tt >= 2:
                        emit_transpose(tt - 2)
                        trans_done += 1

                # Remaining transposes
                for tt in range(trans_done, ntt):
                    emit_transpose(tt)

                # outT[e, t] = Veff.T @ P.T; evict PSUM -> SBUF as f16.
                # (The + xb residual happens on the host in fp32.)
                ob = ostage.tile([128, ET, ntok], F16, tag="ob")
                for e in range(ET):
                    po = ps_out_pool.tile([128, ntok], F32, tag="po")
                    for mt in range(MT):
                        nc.tensor.matmul(
                            po[:],
                            veff_sb[:, mt, ts(e, 128)],
                            pt_sb[:, mt, :],
                            start=(mt == 0), stop=(mt == MT - 1),
                        )
                    if e < DVE_FUSED_TILES:
                        nc.vector.tensor_copy(out=ob[:, e, :], in_=po[:])
                    else:
                        nc.scalar.activation(
                            ob[:, e, :], po[:],
                            mybir.ActivationFunctionType.Copy,
                        )
                    if e == 3:
                        nc.sync.dma_start(
                            outt[c][:, 0:4, bass.ds(toff, ntok)], ob[:, 0:4, :]
                        )
                nc.sync.dma_start(
                    outt[c][:, 4:ET, bass.ds(toff, ntok)], ob[:, 4:ET, :]
                )

            for c in range(N_CHUNKS - 1):
                emit_chunk(c, 0, CHUNK, xt_pre=xt0 if c == 0 else None)
            # Last chunk in two halves: the second half's scores hide the
            # first half's softmax latency at the pipeline tail.
            emit_chunk(N_CHUNKS - 1, 0, CHUNK // 2)
            emit_chunk(N_CHUNKS - 1, CHUNK // 2, CHUNK // 2)

    nc.compile()
    return nc


def _get_nc():
    if "nc" not in _CACHE:
        _CACHE["nc"] = _build_kernel()
    return _CACHE["nc"]


def _pack_x(xb):
    """[T, E] -> [N_CHUNKS, 128, ET, CHUNK] fp16 partition-major chunks."""
    return np.ascontiguousarray(
        xb.reshape(N_CHUNKS, CHUNK, ET, 128).transpose(0, 3, 2, 1),
        dtype=np.float16,
    )


def _pack_rows(w):
    """[R*128, D] -> [128, R, D]: one contiguous run per partition."""
    r = w.shape[0] // 128
    return np.ascontiguousarray(w.reshape(r, 128, -1).transpose(1, 0, 2))


def _unpack_out(o):
    """[N_CHUNKS, 128, ET, CHUNK] -> [T, E] (f16 attn term -> f32)."""
    return o.transpose(0, 3, 2, 1).reshape(T, E).astype(np.float32)


def kernel(x, memory_bank, Wq, bq, Wo, bo):
    global LAST_RESULTS
    x = np.asarray(x, dtype=np.float32)
    mem = np.asarray(memory_bank, dtype=np.float64)
    Wq = np.asarray(Wq, dtype=np.float64)
    bq = np.asarray(bq, dtype=np.float64)
    Wo = np.asarray(Wo, dtype=np.float64)
    bo = np.asarray(bo, dtype=np.float64)

    keff = (mem @ Wq).T                    # [E, M]
    s0 = mem @ bq - bo @ keff              # [M]
    veff = mem @ Wo.T                      # [M, E]

    keff16 = _pack_rows(keff.astype(np.float16))
    veff16 = _pack_rows(veff.astype(np.float16))
    s032 = s0.astype(np.float32).reshape(1, M)
    ident = np.eye(128, dtype=np.float16)
    bo32 = bo.astype(np.float32)

    in_maps = []
    for b in range(B):
        in_maps.append(
            {
                "xbt": _pack_x(x[b] + bo32),
                "keff": keff16,
                "veff": veff16,
                "s0": s032,
                "ones": np.ones((1, 128), dtype=np.float32),
                "ident": ident,
            }
        )

    _ensure_trace_support()
    nc = _get_nc()
    res = bass_utils.run_bass_kernel_spmd(
        nc, in_maps, core_ids=list(range(N_CORES)), trace=TRACE
    )
    LAST_RESULTS = res

    out = np.empty((B, T, E), dtype=np.float32)
    for b in range(B):
        out[b] = _unpack_out(res.results[b]["outt"]) + (x[b] + bo32)
    return out
